# revision 29
# baseline (speedup 1.0000x reference)
"""Multi-head causal attention (B=2, T=2048, D=1024, H=16, Hd=64) on 8 trn2 cores.

Sharding: batch x head-group. Core c handles batch c//4 and heads
(c%4)*4 .. (c%4)*4+3 (data + tensor/head parallel). Each core computes
Q/K/V projections for its 4 heads, causal attention, and a partial
output projection (row-slice of Wo); the host sums the 4 bf16 partials
per batch and adds bo_eff = bo + bv @ Wo (bv commutes through softmax,
so it is folded into the host-side output bias and dropped on-device).

Device layout notes:
- Host passes x^T (q/k/v transposed to [D, T]) in bf16 so every matmul
  contraction has its operand partition-major; no on-chip transposes.
  Projection weights are host-packed to the on-chip [p, k, n] layout so
  their DMAs move 4KB contiguous rows (a strided load runs ~55GB/s and
  would gate the first projection by ~10us).
- Scores are computed transposed (S^T[t2, t1] = K^T.T @ Q^T) so softmax
  sums land on the PE via an appended ones-column in V (row 64 of the
  O^T psum accumulates the denominators for free).
- No max-subtraction in softmax: scaled scores are bounded (|S|/8 < 9
  for N(0,1)-scale inputs; exp stays far from fp32 overflow).
- Normalization: DVE copy of the psum denominator row (the only
  cross-partition hop hardware DVE supports) -> reciprocal -> gpsimd
  partition_broadcast -> DVE multiply into O^T (bf16).

Scheduling notes (HAM clock gate: PE runs 1.2 GHz until ~3.4us of
sustained high-utilization activity, 2.4 GHz after; re-throttles after
an idle window; K=1-contraction matmuls do NOT register as activity):
- The 12.6MB x load is HBM-bandwidth-bound (~36-45us at the per-core
  roofline). x streams as column-halves, earliest-needed-first, over
  the gpsimd/sync/scalar trigger queues; K=128 warmup matmuls (memset
  operands, no DMA deps) bridge until the first slices land.
- Q/K projections are split into 4096-cycle units and dripped into the
  attention streams as PE filler, paced so each unit trails its data's
  arrival; attention chunk 0 runs S-first (all S pairs before any PV)
  so nothing waits on the later x_v stream.
- Output projection runs as [128,512] half-units dripped one chunk
  late, writing bf16 partials (host sums in f32); warm-keeper matmuls
  cover the last norm's DVE/gpsimd latency so the tail stays at 2.4GHz.
"""

import os
import sys

for _p in ("/root/.axon_site/_ro/trn_rl_repo", "/opt/trn_rl_repo"):
    if _p not in sys.path and os.path.isdir(_p):
        sys.path.append(_p)

import numpy as np
import ml_dtypes

B, T, D = 2, 2048, 1024
H, HD = 16, 64
HPC = 4                # heads per core
DH = HPC * HD          # 256 head-dim cols per core
KC = D // 128          # 8 contraction chunks
NT4 = T // 512         # 4 t1-chunks
NB = T // 128          # 16 t2-blocks
N_CORES = 8
N_WARMUP = 40

_BF16 = ml_dtypes.bfloat16
_cache = {}


def _build():
    import concourse.bass as bass
    import concourse.tile as tile
    from concourse import bacc, mybir

    f32 = mybir.dt.float32
    bf16 = mybir.dt.bfloat16
    Exp = mybir.ActivationFunctionType.Exp
    Identity = mybir.ActivationFunctionType.Identity

    nc = bacc.Bacc(target_bir_lowering=False)

    # x^T tensors are host-packed t4-major: [t4, k, p, c] flattened to
    # [NT4*KC*128, 512] so one dma_start covers a full (tensor, t4) slab
    # as a single contiguous-source 1MB transfer (>=75% of DMA peak; the
    # old 2KB-row column slices ran ~107 GB/s/queue).
    xqt_d = nc.declare_dram_parameter("xqt", [NT4 * KC * 128, 512], bf16, isOutput=False)
    xkt_d = nc.declare_dram_parameter("xkt", [NT4 * KC * 128, 512], bf16, isOutput=False)
    xvt_d = nc.declare_dram_parameter("xvt", [NT4 * KC * 128, 512], bf16, isOutput=False)
    wq_d = nc.declare_dram_parameter("wq", [128, KC * DH], bf16, isOutput=False)
    wk_d = nc.declare_dram_parameter("wk", [128, KC * DH], bf16, isOutput=False)
    wv_d = nc.declare_dram_parameter("wv", [128, KC * DH], bf16, isOutput=False)
    wo_d = nc.declare_dram_parameter("wo", [128, 2 * D], bf16, isOutput=False)
    bqk_d = nc.declare_dram_parameter("bqk", [128, 4], f32, isOutput=False)
    tri_d = nc.declare_dram_parameter("tri", [128, 128], bf16, isOutput=False)
    out_d = nc.declare_dram_parameter("out", [T, D], bf16, isOutput=True)

    with tile.TileContext(nc) as tc:
        with tc.tile_pool(name="res", bufs=1) as res, \
             tc.tile_pool(name="ptp", bufs=12) as ptp, \
             tc.tile_pool(name="outp", bufs=3) as outp, \
             tc.tile_pool(name="recp", bufs=2) as recp, \
             tc.tile_pool(name="ps_a", bufs=2, space="PSUM") as ps_a, \
             tc.tile_pool(name="ps_b", bufs=2, space="PSUM") as ps_b, \
             tc.tile_pool(name="ps_o", bufs=1, space="PSUM") as ps_o:

            # ---- warmup operands: no DMA dependency ----
            # K=128 so HAM's activity monitor actually sees the PE as busy
            # (a K=1 warmup exercises 1/128 of the array and reads as idle)
            warm_l = res.tile([128, 128], bf16, name="warm_l")
            warm_r = res.tile([128, 512], bf16, name="warm_r")
            nc.vector.memset(warm_l[:], 0.01)
            nc.vector.memset(warm_r[:], 0.01)

            # ---- persistent tiles ----
            wq_sb = res.tile([128, KC, DH], bf16, name="wq")
            wk_sb = res.tile([128, KC, DH], bf16, name="wk")
            wv_sb = res.tile([128, KC, DH], bf16, name="wv")
            wo_sb = res.tile([128, 2, D], bf16, name="wo")
            bqk_sb = res.tile([128, 4], f32, name="bqk")
            bq_sb = bqk_sb[:, 0:2]
            bk_sb = bqk_sb[:, 2:4]
            tri_sb = res.tile([128, 128], bf16, name="tri")
            xq = res.tile([128, KC, T], bf16, name="xq")
            xk = res.tile([128, KC, T], bf16, name="xk")
            xv = res.tile([128, KC, T], bf16, name="xv")
            qt_sb = [res.tile([128, T], bf16, name=f"qt{i}") for i in range(2)]
            kt_sb = [res.tile([128, T], bf16, name=f"kt{i}") for i in range(2)]
            ont_sb = [res.tile([128, T], bf16, name=f"ont{i}") for i in range(2)]
            # per head: [V_h (64 cols) | ones (64 cols)] — the 64 replicated
            # ones columns make the PV psum accumulate the softmax
            # denominator REPLICATED on psum partitions 64-127, so the norm
            # needs no cross-partition copy and no gpsimd broadcast: recip
            # runs partition-aligned on rows 64-127 and one DVE mul
            # (output-hop, hw-proven) produces the normalized O^T rows.
            vaug_sb = res.tile([128, NB, HPC * (HD + 64)], bf16, name="vaug")
            nc.vector.memset(
                vaug_sb[:].rearrange("p b (h x) -> p b h x", h=HPC)[:, :, :, HD:],
                1.0,
            )

            # ---- DMA triggers: t4-slab granular, earliest-needed-first ----
            # One dma_start per (tensor, t4 quarter): a [128, KC, 512] slab
            # whose DRAM source is fully contiguous (1MB, host-packed
            # t4-major), so each queue streams near its peak. Three queues
            # run concurrently: xq on gpsimd, xk on sync, weights+xv on
            # scalar, late xv slabs spill onto gpsimd/sync after their x
            # streams finish. Slab completion granularity exactly matches
            # unit granularity (a projection unit consumes one (tensor, t4)
            # slab), so compute unblocks as early as possible.
            ge, se, sc = nc.gpsimd, nc.sync, nc.scalar

            def xslab(eng, xd, xt, t4):
                # slab is packed partition-major: per-partition 8KB runs ->
                # 128 descriptors per 1MB DMA. (k-major packing gave 1024
                # 1KB descriptors and the dma_start blocked its issuing
                # engine ~16us on ring backpressure, starving the scalar
                # ACT stream and with it the whole startup.)
                eng.dma_start(
                    out=xt[:, :, t4 * 512 : (t4 + 1) * 512],
                    in_=xd[t4 * KC * 128 : (t4 + 1) * KC * 128, :].rearrange(
                        "(p k) c -> p k c", k=KC
                    ),
                )

            # a dma_start PARKS its issuing engine when the HWDGE ring is
            # full (observed 17-19us for a 3rd+ outstanding 1MB slab).
            # gpsimd/sync have no compute duties, so their parking is free;
            # SCALAR runs the exp/bias ACT stream and must never park —
            # it gets only ring-fitting upfront issues (bqk+tri+wv+xv_t0),
            # and the wo issue is emitted later, inside the chunk-0 code.
            def xslab_half(eng, xd, xt, t4, ph):
                p0, p1 = ph * 64, (ph + 1) * 64
                eng.dma_start(
                    out=xt[p0:p1, :, t4 * 512 : (t4 + 1) * 512],
                    in_=xd[
                        t4 * KC * 128 + p0 * KC : t4 * KC * 128 + p1 * KC, :
                    ].rearrange("(p k) c -> p k c", k=KC),
                )

            sc.dma_start(out=bqk_sb[:], in_=bqk_d[:])
            sc.dma_start(out=tri_sb[:], in_=tri_d[:])
            ge.dma_start(out=wq_sb[:].rearrange("p k n -> p (k n)"), in_=wq_d[:])
            se.dma_start(out=wk_sb[:].rearrange("p k n -> p (k n)"), in_=wk_d[:])
            sc.dma_start(out=wv_sb[:].rearrange("p k n -> p (k n)"), in_=wv_d[:])
            xslab(ge, xqt_d, xq, 0)
            xslab(se, xkt_d, xk, 0)
            xslab(sc, xvt_d, xv, 0)
            xslab(ge, xqt_d, xq, 1)
            xslab(se, xkt_d, xk, 1)
            # xv t1 rides the free queues (~lands 29us for chunk-1 PV
            # b>=4); issuing it from scalar would PARK the scalar engine
            # behind a full HWDGE ring and stall the ACT stream (+7us)
            xslab_half(ge, xvt_d, xv, 1, 0)
            xslab_half(se, xvt_d, xv, 1, 1)
            xslab(ge, xqt_d, xq, 2)
            xslab(se, xkt_d, xk, 2)
            xslab(ge, xqt_d, xq, 3)
            xslab(se, xkt_d, xk, 3)
            xslab(ge, xvt_d, xv, 2)
            xslab(se, xvt_d, xv, 3)
            # landing order (~9.5us/MB/queue):
            #   gpsimd: wq ~5 | xq t0 ~14 | xq t1 ~24 | xv t1a ~29 |
            #           xq t2 ~38 | xq t3 ~47 | xv t2 ~52
            #   sync:   wk ~5 | xk t0 ~14 | xk t1 ~24 | xv t1b ~29 |
            #           xk t2 ~38 | xk t3 ~47 | xv t3 ~52
            #   scalar: bqk tri | wv ~5 | xv t0 ~15 | (wo issued later)

            # ---- PE warmup / filler matmuls ----
            # Dependency-free matmuls keep the PE continuously busy through
            # the DMA-bound startup: without them the data-paced gaps reset
            # HAM's busy window and the whole load phase runs at 1.2 GHz.
            _warm_i = [0]
            def warm(n):
                # alternates between the S-pair ring and the unit ring so
                # consecutive warms land in different psum banks and
                # pipeline instead of serializing on one bank's WAW
                for _ in range(n):
                    _warm_i[0] += 1
                    if _warm_i[0] % 2:
                        wps = ps_a.tile([128, 2, 512], f32, tag="sa", name="warm_ps")
                        out = wps[:, 0, :]
                    else:
                        out = ps_b.tile([128, 512], f32, tag="b", name="warm_ps")[:]
                    nc.tensor.matmul(
                        out, warm_l[:], warm_r[:], start=True, stop=True
                    )
            warm(N_WARMUP)

            # ---- PE filler units -------------------------------------
            def u_qkproj(which, dhc, t4, pad=0):
                # one [128,512] projection unit: q or k, head-dim half dhc,
                # t columns t4*512..+512
                xch, w_sb, b_sb, dst = (
                    (xq, wq_sb, bq_sb, qt_sb) if which == "q" else (xk, wk_sb, bk_sb, kt_sb)
                )
                def emit():
                    ps = ps_b.tile([128, 512], f32, tag="b", name=f"{which}p_ps")
                    for k in range(KC):
                        nc.tensor.matmul(
                            ps[:],
                            w_sb[:, k, dhc * 128 : (dhc + 1) * 128],
                            xch[:, k, t4 * 512 : (t4 + 1) * 512],
                            start=(k == 0),
                            stop=(k == KC - 1),
                        )
                        if pad:
                            warm(pad)
                    nc.scalar.activation(
                        out=dst[dhc][:, t4 * 512 : (t4 + 1) * 512],
                        in_=ps[:],
                        func=Identity,
                        bias=b_sb[:, dhc : dhc + 1],
                        scale=1.0,
                    )
                return emit

            def u_vproj(tb):
                def emit():
                    ps = ps_b.tile([128, 512], f32, tag="b", name="v_ps")
                    for k in range(KC):
                        nc.tensor.matmul(
                            ps[:, 0:DH],
                            xv[:, k, tb * 128 : (tb + 1) * 128],
                            wv_sb[:, k, :],
                            start=(k == 0),
                            stop=(k == KC - 1),
                        )
                    nc.vector.tensor_copy(
                        out=vaug_sb[:, tb, :].rearrange("p (h x) -> p h x", h=HPC)[:, :, 0:HD],
                        in_=ps[:, 0:DH].rearrange("p (h x) -> p h x", h=HPC),
                    )
                return emit

            def u_oproj(m, n2, ob_ref):
                # half output-projection unit: out columns n2*512..+512 of
                # t1 block m. ob_ref[0] holds the bf16 staging tile shared
                # by the two halves of block m. Each half DMAs out on its
                # own queue (round-robin) right after its cast, so the last
                # block's writeback doesn't serialize behind 3 earlier
                # 256KB transfers on one queue (~5us exposed tail before).
                def emit():
                    if n2 == 0:
                        ob_ref[0] = outp.tile([128, D], bf16, tag="ob", name="ob")
                    ps = ps_b.tile([128, 512], f32, tag="b", name="op_ps")
                    for dhc in range(2):
                        nc.tensor.matmul(
                            ps[:],
                            ont_sb[dhc][:, m * 128 : (m + 1) * 128],
                            wo_sb[:, dhc, n2 * 512 : (n2 + 1) * 512],
                            start=(dhc == 0),
                            stop=(dhc == 1),
                        )
                    ob = ob_ref[0]
                    nc.vector.tensor_copy(out=ob[:, n2 * 512 : (n2 + 1) * 512], in_=ps[:])
                    eng = (nc.sync, nc.gpsimd, nc.scalar)[(2 * m + n2) % 3]
                    eng.dma_start(
                        out=out_d[m * 128 : (m + 1) * 128, n2 * 512 : (n2 + 1) * 512],
                        in_=ob[:, n2 * 512 : (n2 + 1) * 512],
                    )
                return emit

            def ops_for(c):
                units = []
                for m in range(4 * c, 4 * c + 4):
                    ob_ref = [None]
                    units.append(u_oproj(m, 0, ob_ref))
                    units.append(u_oproj(m, 1, ob_ref))
                return units

            # ---- attention streams ----------------------------------
            # S^T tiles are head-PAIRED: heads 2p (rows 0-63 of kt/qt
            # tile p) and 2p+1 (rows 64-127) are emitted back-to-back, so
            # their 64-contraction matmuls land in disjoint PE row-groups
            # (tile_position (0,0) / (64,0) auto-derived) and execute
            # CONCURRENTLY — the S phase runs at ~2x the serial rate.
            # Both slots of a pair-tile share one block b, so the causal
            # geometry matches and a single merged exp covers both.
            def make_spair2(p, c):
                def s_pair(b):
                    s_ps = ps_a.tile([128, 2, 512], f32, tag="sa", name="s_ps")
                    pt = ptp.tile([128, 2, 512], bf16, tag="pt", name="pt")
                    r = b - 4 * c
                    off = max(r, 0) * 128
                    w = 512 - off
                    for i in range(2):
                        hr = i * 64
                        nc.tensor.matmul(
                            s_ps[:, i, off : off + w],
                            kt_sb[p][hr : hr + 64, b * 128 : (b + 1) * 128],
                            qt_sb[p][hr : hr + 64, c * 512 + off : (c + 1) * 512],
                            start=True,
                            stop=True,
                        )
                    if r < 0:
                        nc.scalar.activation(out=pt[:], in_=s_ps[:], func=Exp, scale=0.125)
                    else:
                        nc.scalar.activation(
                            out=pt[:, :, off : off + w],
                            in_=s_ps[:, :, off : off + w],
                            func=Exp,
                            scale=0.125,
                        )
                        for i in range(2):
                            nc.vector.tensor_mul(
                                pt[:, i, off : off + 128],
                                pt[:, i, off : off + 128],
                                tri_sb[:],
                            )
                    return (pt, (b, r, off, w))
                return s_pair

            def pv_block(h, c, b, tiles, o_ps, nblk):
                pt, (_, r, off, w) = tiles[b]
                i = h % 2
                nc.tensor.matmul(
                    o_ps[:, off : off + w],
                    vaug_sb[:, b, h * 128 : (h + 1) * 128],
                    pt[:, i, off : off + w],
                    start=(b == 0),
                    stop=(b == nblk - 1),
                )

            def norm(h, c, o_ps):
                # denominator arrives replicated on psum rows 64-127 (the 64
                # ones-columns in vaug), so recip is fully partition-aligned;
                # the mul's output-partition hop is the hw-proven DVE case.
                hc, hr = h // 2, (h % 2) * 64
                den = recp.tile([64, 512], f32, tag="den", name="den")
                nc.vector.tensor_copy(out=den[:], in_=o_ps[HD:128, :])
                rec = recp.tile([64, 512], f32, tag="rec", name="rec")
                nc.vector.reciprocal_approx_fast(out=rec[:], in_=den[:])
                nc.vector.tensor_mul(
                    ont_sb[hc][hr : hr + 64, c * 512 : (c + 1) * 512],
                    o_ps[0:HD, :],
                    rec[:],
                )

            # prelude: the two units attention chunk 0 needs (t0 slabs land
            # atomically, so no intra-unit padding — the warm block above
            # bridges until the first slab+weights arrive)
            u_qkproj("q", 0, 0)()
            u_qkproj("k", 0, 0)()

            # per-chunk drip units, ordered to respect dependencies:
            #   - v(4c..4c+3) precede PV(0, those blocks)  -> stream A slots
            #   - qk(1,0) precede S(2) of chunk 0          -> head-0 round
            #   - qk(*,t) precede chunk t's streams        -> chunk t-1
            #   - op halves of chunk c-1 run in chunk c (norms done)
            dripA = {
                1: [u_vproj(4), u_vproj(5), u_vproj(6), u_vproj(7)],
                2: [u_vproj(8), u_vproj(9), u_vproj(10), u_vproj(11)],
                3: [u_vproj(12), u_vproj(13), u_vproj(14), u_vproj(15)],
            }
            v_c0 = [u_vproj(0), u_vproj(1), u_vproj(2), u_vproj(3)]
            qk_c0_s = [u_qkproj("q", 1, 0), u_qkproj("k", 1, 0)]  # t0 data only
            qk_c0_pv = [u_qkproj("q", 0, 1), u_qkproj("k", 0, 1)] # t1 arrives ~25-28us
            _ops0 = ops_for(0)
            dripH = {
                1: _ops0[:2] + [u_qkproj("q", 1, 1), u_qkproj("k", 1, 1)]
                   + _ops0[2:]
                   + [u_qkproj("q", 0, 2), u_qkproj("k", 0, 2),
                      u_qkproj("q", 1, 2), u_qkproj("k", 1, 2)],
                2: ops_for(1) + [u_qkproj("q", 0, 3), u_qkproj("k", 0, 3),
                                 u_qkproj("q", 1, 3), u_qkproj("k", 1, 3)],
                3: ops_for(2),
            }

            if True:
                # ---- chunk 0: S-first schedule ----
                # All S pair-tiles + t0-based drips run before any PV so
                # nothing waits on x_v; t1 projection units sit in the PV
                # section (their slices arrive ~25-28us, in time for c=1).
                c, nblk = 0, 4
                o_pss = [
                    ps_o.tile([128, 512], f32, tag=f"ops{h % 2}", name=f"ops{h}")
                    for h in range(HPC)
                ]
                sp0, sp1 = make_spair2(0, 0), make_spair2(1, 0)
                pts0 = [sp0(0), sp0(1)]
                qk_c0_s[0]()
                pts0.append(sp0(2))
                qk_c0_s[1]()                 # q/k dhc1 before pair-1 S
                pts0.append(sp0(3))
                pts1 = [sp1(0), sp1(1)]
                v_c0[0]()
                pts1.append(sp1(2))
                v_c0[1]()
                pts1.append(sp1(3))
                ptss = {0: pts0, 1: pts1}
                qk_c0_pv[0]()                # q dhc0 t1 (lands ~24us)
                pv_block(0, 0, 0, pts0, o_pss[0], nblk)
                pv_block(0, 0, 1, pts0, o_pss[0], nblk)
                for u in (v_c0[2], v_c0[3]):
                    u()
                pv_block(0, 0, 2, pts0, o_pss[0], nblk)
                pv_block(0, 0, 3, pts0, o_pss[0], nblk)
                norm(0, 0, o_pss[0])
                sc.dma_start(out=wo_sb[:].rearrange("p c n -> p (c n)"), in_=wo_d[:])
                for h in range(1, HPC):
                    for b in range(nblk):
                        pv_block(h, 0, b, ptss[h // 2], o_pss[h], nblk)
                    if h == 1:
                        qk_c0_pv[1]()        # k dhc0 t1 (lands ~24us)
                    norm(h, 0, o_pss[h])
                ptss.clear()

            LAG = 4
            for c in range(1, NT4):
                nblk = 4 * c + 4
                da = list(dripA[c])
                dh = list(dripH[c])

                o_pss = [
                    ps_o.tile([128, 512], f32, tag=f"ops{h % 2}", name=f"ops{h}")
                    for h in range(HPC)
                ]

                # two half-chunk phases: phase p covers heads 2p, 2p+1.
                # S pair-tiles run LAG blocks ahead of the two PV streams so
                # exp latency stays off the PE critical path while only
                # LAG+2 pt tiles are ever live. vproj drips (da) go first —
                # this chunk's own PV(b>=4) consumes their vaug blocks;
                # oproj/qkproj drips (dh) spread over the remaining slots.
                for p in range(2):
                    sp = make_spair2(p, c)
                    tiles = [None] * nblk
                    nslot = nblk + LAG
                    ndrip_p = len(da) + ((len(dh) + 1) // 2 if p == 0 else len(dh))
                    done = 0
                    for s in range(nslot):
                        if s < nblk:
                            tiles[s] = sp(s)
                        bb = s - LAG
                        if bb >= 0:
                            pv_block(2 * p, c, bb, tiles, o_pss[2 * p], nblk)
                            pv_block(2 * p + 1, c, bb, tiles, o_pss[2 * p + 1], nblk)
                        want = (ndrip_p * (s + 1)) // nslot
                        while done < want:
                            (da if da else dh).pop(0)()
                            done += 1
                    norm(2 * p, c, o_pss[2 * p])
                    norm(2 * p + 1, c, o_pss[2 * p + 1])
                while dh:
                    dh.pop(0)()

            # keep the PE busy across the last norm's DVE/gpsimd latency
            # so HAM stays un-throttled for the final output projections
            warm(18)

            # final chunk's output projection; warm fillers absorb the
            # CAST-paced bubbles of the 2-buffer psum ring so the tail
            # never re-throttles
            _final = ops_for(3)
            for i, u in enumerate(_final):
                u()
                if i < len(_final) - 1:
                    warm(1)

    nc.compile()
    return nc


def _get_nc():
    if "nc" not in _cache:
        _cache["nc"] = _build()
    return _cache["nc"]


def build_in_maps(query, key, value, Wq, bq, Wk, bk, Wv, bv, Wo, bo):
    query = np.asarray(query, np.float32)
    key = np.asarray(key, np.float32)
    value = np.asarray(value, np.float32)
    Wq_, Wk_, Wv_, Wo_ = (np.asarray(a, np.float32) for a in (Wq, Wk, Wv, Wo))
    bq_, bk_, bv_, bo_ = (np.asarray(a, np.float32) for a in (bq, bk, bv, bo))

    def pack_x(x):
        # [T, D] -> x^T tiled [t4, p, k, c] -> [NT4*128*KC, 512]: each
        # (t4) slab is 1MB contiguous with 8KB per-partition runs, so one
        # dma_start covers it with only 128 descriptors
        xt = x.T.reshape(KC, 128, NT4, 512).transpose(2, 1, 0, 3)
        return np.ascontiguousarray(xt.reshape(NT4 * KC * 128, 512)).astype(_BF16)

    xqt = [pack_x(query[b]) for b in range(B)]
    xkt = [pack_x(key[b]) for b in range(B)]
    xvt = [pack_x(value[b]) for b in range(B)]

    tri = np.tril(np.ones((128, 128), np.float32)).T.astype(_BF16)  # tri[j,i]=1 iff j<=i

    # bv commutes through softmax: fold it into the output bias
    bo_eff = (bo_ + bv_ @ Wo_).astype(np.float32)

    def pack_w(w):  # [KC*128, N] -> [128, KC*N] matching sbuf [p, k, n]
        kc, n = w.shape[0] // 128, w.shape[1]
        return np.ascontiguousarray(
            w.reshape(kc, 128, n).transpose(1, 0, 2).reshape(128, kc * n)
        ).astype(_BF16)

    in_maps = []
    for c in range(N_CORES):
        b, hg = c // 4, c % 4
        sl = slice(hg * DH, (hg + 1) * DH)
        bqk = np.stack(
            [bq_[sl][0:128], bq_[sl][128:256], bk_[sl][0:128], bk_[sl][128:256]],
            axis=1,
        )
        in_maps.append(
            {
                "xqt": xqt[b],
                "xkt": xkt[b],
                "xvt": xvt[b],
                "wq": pack_w(Wq_[:, sl]),
                "wk": pack_w(Wk_[:, sl]),
                "wv": pack_w(Wv_[:, sl]),
                "wo": pack_w(Wo_[sl, :]),
                "bqk": np.ascontiguousarray(bqk, np.float32),
                "tri": tri,
            }
        )

    return in_maps, bo_eff


def kernel(query, key, value, Wq, bq, Wk, bk, Wv, bv, Wo, bo):
    from concourse.bass_utils import run_bass_kernel_spmd

    nc = _get_nc()
    in_maps, bo_eff = build_in_maps(query, key, value, Wq, bq, Wk, bk, Wv, bv, Wo, bo)
    res = run_bass_kernel_spmd(nc, in_maps, list(range(N_CORES)))
    _cache["last_results"] = res

    out = np.empty((B, T, D), np.float32)
    for b in range(B):
        acc = res.results[4 * b]["out"].astype(np.float32)
        for hg in range(1, 4):
            acc = acc + res.results[4 * b + hg]["out"].astype(np.float32)
        out[b] = acc + bo_eff[None, :]
    return out

